# revision 15
# baseline (speedup 1.0000x reference)
"""Trainium2 Bass kernel for nn_CEBlock (transformer block + attention-derived
GCN layer), data-parallel over 8 NeuronCores.

Self-contained: hardcodes all shapes; accepts the full unsharded inputs and
returns the full output.
"""
import sys

if "/opt/trn_rl_repo" not in sys.path:
    sys.path.insert(0, "/opt/trn_rl_repo")

import numpy as np
import ml_dtypes

import concourse.bacc as bacc
import concourse.mybir as mybir
import concourse.tile as tile
from concourse import bass

F32 = mybir.dt.float32
BF16 = mybir.dt.bfloat16

# Problem constants
B, N, C = 32, 320, 768
H, HD = 12, 64
LT, LS = 64, 256
MLP_H = 3072
EPS = 1e-5
N_CORES = 8
B_LOC = B // N_CORES

P = 128
KC = C // P          # 6 c-chunks
FC = MLP_H // P      # 24 f-chunks
MC_QK = (2 * C) // P  # 12 chunks for q+k outputs
# token chunks (natural) and shifted chunks (for GCN output)
TCH = [(0, 128), (128, 128), (256, 64)]
SH = [(0, 64), (64, 128), (192, 128)]
KSZ = [128, 128, 64]  # key chunk sizes (same as TCH sizes)

Gelu = mybir.ActivationFunctionType.Gelu
Exp = mybir.ActivationFunctionType.Exp
Ln = mybir.ActivationFunctionType.Ln
Identity = mybir.ActivationFunctionType.Identity
SUB = mybir.AluOpType.subtract
MULT = mybir.AluOpType.mult
ADD = mybir.AluOpType.add


def build_nc(b_loc=B_LOC, flags=None):
    """Build the per-core Bass program. Same program on all 8 cores (SPMD)."""
    fl = flags or {}
    nc = bacc.Bacc("TRN2", target_bir_lowering=False, debug=True)

    # ---- DRAM parameters -------------------------------------------------
    x_e = nc.dram_tensor("x", (b_loc, N, C), F32, kind="ExternalInput")
    wqkv_e = nc.dram_tensor("wqkv", (C, 3 * C), BF16, kind="ExternalInput")
    wproj_e = nc.dram_tensor("wproj", (C, C), BF16, kind="ExternalInput")
    w1_e = nc.dram_tensor("w1", (C, MLP_H), BF16, kind="ExternalInput")
    w2_e = nc.dram_tensor("w2", (MLP_H, C), BF16, kind="ExternalInput")
    wg_e = nc.dram_tensor("wg", (C, C), BF16, kind="ExternalInput")
    bexp_e = nc.dram_tensor("bexp", (H, C), BF16, kind="ExternalInput")
    bsel_e = nc.dram_tensor("bsel", (H, H * P), BF16, kind="ExternalInput")
    id32_e = nc.dram_tensor("id32", (P, P), F32, kind="ExternalInput")
    id16_e = nc.dram_tensor("id16", (P, P), BF16, kind="ExternalInput")
    out_e = nc.dram_tensor("out", (b_loc, N, C), F32, kind="ExternalOutput")

    has_aff3 = fl.get("has_aff3", False)
    if has_aff3:
        g3_e = nc.dram_tensor("g3v", (C,), F32, kind="ExternalInput")
        b3_e = nc.dram_tensor("b3v", (C,), F32, kind="ExternalInput")
    has_bias = fl.get("has_bias", False)
    if has_bias:
        # packed per-partition biases: qkv [128,12], h [128,24] ; row biases
        bqkv_e = nc.dram_tensor("bqkv2", (P, MC_QK), F32, kind="ExternalInput")
        bm1_e = nc.dram_tensor("bm12", (P, FC), F32, kind="ExternalInput")
        bv_e = nc.dram_tensor("bvv", (C,), F32, kind="ExternalInput")
        bproj_e = nc.dram_tensor("bprojv", (C,), F32, kind="ExternalInput")
        bm2_e = nc.dram_tensor("bm2v", (C,), F32, kind="ExternalInput")
        bg_e = nc.dram_tensor("bgv", (C,), F32, kind="ExternalInput")

    with tile.TileContext(nc) as tc:
        with (
            tc.tile_pool(name="wp", bufs=1) as wp,
            tc.tile_pool(name="act", bufs=1) as actp,
            tc.tile_pool(name="st", bufs=1) as stp,
            tc.tile_pool(name="ps", bufs=2, space="PSUM") as psp,
            tc.tile_pool(name="ps1", bufs=3, space="PSUM") as ps1p,
        ):
            # ---- resident weights -----------------------------------
            wqkv = wp.tile([P, KC, 3 * C], BF16)
            nc.sync.dma_start(wqkv[:], wqkv_e[:].rearrange("(kc p) m -> p kc m", p=P))
            wproj = wp.tile([P, KC, C], BF16)
            nc.sync.dma_start(wproj[:], wproj_e[:].rearrange("(kc p) m -> p kc m", p=P))
            w1 = wp.tile([P, KC, MLP_H], BF16)
            nc.sync.dma_start(w1[:], w1_e[:].rearrange("(kc p) m -> p kc m", p=P))
            w2 = wp.tile([P, FC, C], BF16)
            nc.sync.dma_start(w2[:], w2_e[:].rearrange("(fc p) m -> p fc m", p=P))
            wg = wp.tile([P, KC, C], BF16)
            nc.sync.dma_start(wg[:], wg_e[:].rearrange("(kc p) m -> p kc m", p=P))
            bexp = wp.tile([H, KC, P], BF16)
            nc.sync.dma_start(bexp[:], bexp_e[:].rearrange("h (kc p) -> h kc p", p=P))
            bsel = wp.tile([H, H, P], BF16)
            nc.sync.dma_start(bsel[:], bsel_e[:].rearrange("a (h p) -> a h p", p=P))
            id32 = wp.tile([P, P], F32)
            nc.sync.dma_start(id32[:], id32_e[:])
            id16 = wp.tile([P, P], BF16)
            nc.sync.dma_start(id16[:], id16_e[:])
            eps_t = wp.tile([P, 1], F32)
            nc.vector.memset(eps_t[:], EPS)
            ones1 = wp.tile([1, P], BF16)
            nc.vector.memset(ones1[:], 1.0)
            if has_aff3:
                g3b = wp.tile([P, C], F32)
                nc.sync.dma_start(g3b[:], g3_e[None, :].to_broadcast((P, C)))
                b3b = wp.tile([P, C], F32)
                nc.sync.dma_start(b3b[:], b3_e[None, :].to_broadcast((P, C)))
            if has_bias:
                bqkv = wp.tile([P, MC_QK], F32)
                nc.sync.dma_start(bqkv[:], bqkv_e[:])
                bm1 = wp.tile([P, FC], F32)
                nc.sync.dma_start(bm1[:], bm1_e[:])
                bvb = wp.tile([P, C], F32)
                nc.sync.dma_start(bvb[:], bv_e[None, :].to_broadcast((P, C)))
                bprojb = wp.tile([P, C], F32)
                nc.sync.dma_start(bprojb[:], bproj_e[None, :].to_broadcast((P, C)))
                bm2b = wp.tile([P, C], F32)
                nc.sync.dma_start(bm2b[:], bm2_e[None, :].to_broadcast((P, C)))
                bgb = wp.tile([P, C], F32)
                nc.sync.dma_start(bgb[:], bg_e[None, :].to_broadcast((P, C)))

            # ================= per-sample loop ========================
            for s in range(b_loc):
                # ---- load x (token-major) ---------------------------
                x_sb = actp.tile([P, 3, C], F32, tag="resid", bufs=2)
                nc.sync.dma_start(
                    x_sb[:, 0:2, :],
                    x_e[s, 0:256, :].rearrange("(t p) c -> p t c", p=P),
                )
                nc.sync.dma_start(x_sb[0:64, 2, :], x_e[s, 256:320, :])

                def layernorm_to(src, chunks, out_tile, out_chunk_idx=None):
                    """LN over C for token-major src([P,3,C] or [p,C] chunks);
                    returns (mv3, rstd3) and writes normalized bf16 to out_tile."""
                    st6 = stp.tile([P, 3, 3, 6], F32, tag="st6")
                    mv3 = stp.tile([P, 3, 2], F32, tag="mv3")
                    nc.vector.memset(mv3[:], 1.0)
                    for tcidx, (ts, tsz) in enumerate(chunks):
                        for sub in range(3):
                            nc.vector.bn_stats(
                                out=st6[0:tsz, tcidx, sub, :],
                                in_=src[0:tsz, tcidx, sub * 256:(sub + 1) * 256],
                            )
                        nc.vector.bn_aggr(
                            out=mv3[0:tsz, tcidx, :], in_=st6[0:tsz, tcidx, :, :]
                        )
                    lnv = stp.tile([P, 3], F32, tag="lnv")
                    nc.scalar.activation(
                        out=lnv[:], in_=mv3[:, :, 1], func=Ln, bias=eps_t[:],
                        scale=1.0,
                    )
                    rstd3 = stp.tile([P, 3], F32, tag="rstd3")
                    nc.scalar.activation(
                        out=rstd3[:], in_=lnv[:], func=Exp, bias=0.0, scale=-0.5
                    )
                    for tcidx, (ts, tsz) in enumerate(chunks):
                        nc.vector.tensor_scalar(
                            out=out_tile[0:tsz, tcidx, :],
                            in0=src[0:tsz, tcidx, :],
                            scalar1=mv3[0:tsz, tcidx, 0:1],
                            scalar2=rstd3[0:tsz, tcidx:tcidx + 1],
                            op0=SUB,
                            op1=MULT,
                        )
                    return mv3, rstd3

                def transpose_tm_to_fm(src_tm, dt, ident, out_fm):
                    """src_tm [P,3,C'] (token-major, dtype dt) -> out_fm
                    [P, C'//P, 320] via PE transposes."""
                    nch = src_tm.shape[2] // P
                    for kc in range(nch):
                        pt = ps1p.tile([P, 512], dt, tag="one")
                        for tcidx, (ts, tsz) in enumerate(TCH):
                            nc.tensor.transpose(
                                pt[:, ts:ts + tsz],
                                src_tm[0:tsz, tcidx, kc * P:(kc + 1) * P],
                                ident[0:tsz, 0:tsz],
                            )
                        nc.any.tensor_copy(out=out_fm[:, kc, :], in_=pt[:, 0:N])

                # ---- LN1 + transpose --------------------------------
                xn_tm = actp.tile([P, 3, C], BF16, tag="xn")
                layernorm_to(x_sb, TCH, xn_tm)
                xnT = actp.tile([P, KC, N], BF16, tag="actT")
                transpose_tm_to_fm(xn_tm, BF16, id16, xnT)

                # ---- QKV: q,k feature-major -------------------------
                qk = actp.tile([P, MC_QK, N], BF16, tag="qk")
                for mc in range(MC_QK):
                    pq = ps1p.tile([P, 512], F32, tag="one")
                    for kc in range(KC):
                        nc.tensor.matmul(
                            pq[:, 0:N],
                            lhsT=wqkv[:, kc, mc * P:(mc + 1) * P],
                            rhs=xnT[:, kc, :],
                            start=(kc == 0),
                            stop=(kc == KC - 1),
                        )
                    if has_bias:
                        nc.vector.tensor_scalar(
                            out=qk[:, mc, :], in0=pq[:, 0:N],
                            scalar1=bqkv[:, mc:mc + 1], scalar2=None, op0=ADD,
                        )
                    else:
                        nc.any.tensor_copy(out=qk[:, mc, :], in_=pq[:, 0:N])

                # ---- V token-major (augmented with ones col) --------
                v_sb = actp.tile([P, 3, H, HD + 1], BF16, tag="vsm")
                for tcidx, (ts, tsz) in enumerate(TCH):
                    for half in range(2):
                        pv = ps1p.tile([P, 512], F32, tag="one")
                        for kc in range(KC):
                            nc.tensor.matmul(
                                pv[0:tsz, 0:384],
                                lhsT=xnT[:, kc, ts:ts + tsz],
                                rhs=wqkv[:, kc, 2 * C + half * 384:2 * C + (half + 1) * 384],
                                start=(kc == 0),
                                stop=(kc == KC - 1),
                            )
                        if has_bias:
                            nc.vector.tensor_add(
                                out=pv[0:tsz, 0:384], in0=pv[0:tsz, 0:384],
                                in1=bvb[0:tsz, half * 384:(half + 1) * 384],
                            )
                        nc.any.tensor_copy(
                            out=v_sb[0:tsz, tcidx, half * 6:(half + 1) * 6, 0:HD],
                            in_=pv[0:tsz, 0:384].rearrange("p (h d) -> p h d", d=HD),
                        )
                    nc.vector.memset(v_sb[0:tsz, tcidx, :, HD:HD + 1], 1.0)

                # ---- attention logits + exp (key-major) -------------
                probs = actp.tile([P, 3, H, N], BF16, tag="big")
                for j, kz in enumerate(KSZ):
                    for g in range(H // 2):
                        pl = psp.tile([P, 2, 512], F32, tag="pair")
                        for hh in range(2):
                            h = 2 * g + hh
                            po = (h % 2) * HD
                            nc.tensor.matmul(
                                pl[0:kz, hh, 0:N],
                                lhsT=qk[po:po + HD, 6 + h // 2, TCH[j][0]:TCH[j][0] + kz],
                                rhs=qk[po:po + HD, h // 2, :],
                                start=True,
                                stop=True,
                            )
                        nc.scalar.activation(
                            out=probs[0:kz, j, 2 * g:2 * g + 2, :],
                            in_=pl[0:kz, :, 0:N],
                            func=Exp,
                        )

                # ---- ctx (feature-major) + denominators -------------
                ctx = actp.tile([P, KC, N], BF16, tag="ctx")
                dstage = stp.tile([H, N], F32, tag="dstage")
                for h in range(H):
                    pc = ps1p.tile([HD + 1, 512], F32, tag="one")
                    for j, kz in enumerate(KSZ):
                        nc.tensor.matmul(
                            pc[:, 0:N],
                            lhsT=v_sb[0:kz, j, h, :],
                            rhs=probs[0:kz, j, h, :],
                            start=(j == 0),
                            stop=(j == 2),
                        )
                    nc.vector.tensor_copy(
                        out=ctx[(h % 2) * HD:(h % 2) * HD + HD, h // 2, :],
                        in_=pc[0:HD, 0:N],
                    )
                    # stage at base-64 (quadrant-legal), then DMA-repack
                    d64 = stp.tile([HD + 1, N], F32, tag="d64", bufs=2)
                    nc.vector.tensor_copy(
                        out=d64[HD:HD + 1, :], in_=pc[HD:HD + 1, 0:N],
                    )
                    nc.sync.dma_start(dstage[h:h + 1, :], d64[HD:HD + 1, :])
                nc.scalar.activation(out=dstage[:], in_=dstage[:], func=Ln)
                recip = stp.tile([H, N], BF16, tag="recip")
                nc.scalar.activation(out=recip[:], in_=dstage[:], func=Exp, scale=-1.0)

                # normalize ctx via broadcast grid, cast to bf16
                ctxn = actp.tile([P, KC, N], BF16, tag="ctxn")
                for kc in range(KC):
                    pg = ps1p.tile([P, 512], F32, tag="one")
                    nc.tensor.matmul(
                        pg[:, 0:N], lhsT=bexp[:, kc, :], rhs=recip[:],
                        start=True, stop=True,
                    )
                    nc.vector.tensor_mul(
                        out=ctxn[:, kc, :], in0=ctx[:, kc, :], in1=pg[:, 0:N]
                    )

                # ---- w_ts (adjacency) -------------------------------
                # Rk: broadcast each head's recip row across all partitions
                # via one-hot selector matmuls (out[p,i] = recip[h,i])
                prk = psp.tile([P, 2, 512], F32, tag="pair")
                for h in range(H):
                    fo = h * HD  # flat col offset: bank-safe since 64 | 512
                    nc.tensor.matmul(
                        prk[:, fo // 512, fo % 512:fo % 512 + HD],
                        lhsT=bsel[:, h, :],
                        rhs=recip[:, 0:LT],
                        start=True, stop=True,
                    )
                prk_v = prk[:].rearrange("p a b -> p (a b)")[:, 0:H * HD].rearrange(
                    "p (h i) -> p h i", i=HD)
                pw = ps1p.tile([64, 512], F32, tag="one")
                wtmp = stp.tile([P, H, LT], BF16, tag="wtmp")
                wpiece = stp.tile([P, 3, LT], F32, tag="wpiece")
                for j, (off, kz) in enumerate([(64, 64), (0, 128), (0, 64)]):
                    nc.vector.tensor_tensor(
                        out=wtmp[0:kz, :, :],
                        in0=probs[off:off + kz, j, :, 0:LT],
                        in1=prk_v[0:kz, :, :],
                        op=MULT,
                    )
                    nc.vector.tensor_reduce(
                        out=wpiece[0:kz, j, :],
                        in_=wtmp[0:kz, :, :].rearrange("p h i -> p i h"),
                        axis=mybir.AxisListType.X, op=ADD,
                    )
                # transpose pieces into [64, 256] (search-major -> template-major)
                nc.tensor.transpose(pw[:, 0:64], wpiece[0:64, 0, :], id32[0:64, 0:64])
                nc.tensor.transpose(pw[:, 64:192], wpiece[0:128, 1, :], id32[:, :])
                nc.tensor.transpose(pw[:, 192:256], wpiece[0:64, 2, :], id32[0:64, 0:64])
                wts_e = stp.tile([LT, LS], BF16, tag="wts_e")
                wden = stp.tile([LT, 1], F32, tag="wden")
                nc.scalar.activation(
                    out=wts_e[:], in_=pw[:, 0:LS], func=Exp, scale=1.0 / H,
                    accum_out=wden[:],
                )
                wrec = stp.tile([LT, 1], F32, tag="wrec")
                nc.vector.reciprocal(out=wrec[:], in_=wden[:])
                wts = stp.tile([LT, LS], BF16, tag="wts")
                nc.vector.tensor_scalar_mul(wts[:], wts_e[:], wrec[:])
                wtsT = stp.tile([P, 2, LT], BF16, tag="wtsT")
                for mm in range(2):
                    pwt = ps1p.tile([P, 512], BF16, tag="onebf", bufs=1)
                    nc.tensor.transpose(
                        pwt[:, 0:LT], wts[:, mm * P:(mm + 1) * P], id16[0:LT, 0:LT]
                    )
                    nc.any.tensor_copy(out=wtsT[:, mm, :], in_=pwt[:, 0:LT])

                # ---- proj + residual -> x1 --------------------------
                x1_sb = actp.tile([P, 3, C], F32, tag="resid", bufs=2)
                for tcidx, (ts, tsz) in enumerate(TCH):
                    for half in range(2):
                        pp = ps1p.tile([P, 512], F32, tag="one")
                        for kc in range(KC):
                            nc.tensor.matmul(
                                pp[0:tsz, 0:384],
                                lhsT=ctxn[:, kc, ts:ts + tsz],
                                rhs=wproj[:, kc, half * 384:(half + 1) * 384],
                                start=(kc == 0),
                                stop=(kc == KC - 1),
                            )
                        if has_bias:
                            nc.vector.tensor_add(
                                out=pp[0:tsz, 0:384], in0=pp[0:tsz, 0:384],
                                in1=bprojb[0:tsz, half * 384:(half + 1) * 384],
                            )
                        nc.vector.tensor_add(
                            out=x1_sb[0:tsz, tcidx, half * 384:(half + 1) * 384],
                            in0=x_sb[0:tsz, tcidx, half * 384:(half + 1) * 384],
                            in1=pp[0:tsz, 0:384],
                        )

                # ---- LN2 + transpose --------------------------------
                xn2_tm = actp.tile([P, 3, C], BF16, tag="xn")
                layernorm_to(x1_sb, TCH, xn2_tm)
                xn2T = actp.tile([P, KC, N], BF16, tag="actT")
                transpose_tm_to_fm(xn2_tm, BF16, id16, xn2T)

                # ---- MLP1 + gelu (h feature-major) ------------------
                h_sb = actp.tile([P, FC, N], BF16, tag="big")
                for g in range(FC // 2):
                    ph = psp.tile([P, 2, 512], F32, tag="pair")
                    for ff in range(2):
                        fc = 2 * g + ff
                        for kc in range(KC):
                            nc.tensor.matmul(
                                ph[:, ff, 0:N],
                                lhsT=w1[:, kc, fc * P:(fc + 1) * P],
                                rhs=xn2T[:, kc, :],
                                start=(kc == 0),
                                stop=(kc == KC - 1),
                            )
                    if has_bias:
                        for ff in range(2):
                            nc.scalar.activation(
                                out=h_sb[:, 2 * g + ff, :], in_=ph[:, ff, 0:N],
                                func=Gelu, bias=bm1[:, 2 * g + ff:2 * g + ff + 1],
                                scale=1.0,
                            )
                    else:
                        nc.scalar.activation(
                            out=h_sb[:, 2 * g:2 * g + 2, :], in_=ph[:, :, 0:N],
                            func=Gelu,
                        )

                # ---- MLP2 + residual -> x2 --------------------------
                x2_sb = actp.tile([P, 3, C], F32, tag="resid", bufs=2)
                for tcidx, (ts, tsz) in enumerate(TCH):
                    for half in range(2):
                        pm = ps1p.tile([P, 512], F32, tag="one")
                        for fc in range(FC):
                            nc.tensor.matmul(
                                pm[0:tsz, 0:384],
                                lhsT=h_sb[:, fc, ts:ts + tsz],
                                rhs=w2[:, fc, half * 384:(half + 1) * 384],
                                start=(fc == 0),
                                stop=(fc == FC - 1),
                            )
                        if has_bias:
                            nc.vector.tensor_add(
                                out=pm[0:tsz, 0:384], in0=pm[0:tsz, 0:384],
                                in1=bm2b[0:tsz, half * 384:(half + 1) * 384],
                            )
                        nc.vector.tensor_add(
                            out=x2_sb[0:tsz, tcidx, half * 384:(half + 1) * 384],
                            in0=x1_sb[0:tsz, tcidx, half * 384:(half + 1) * 384],
                            in1=pm[0:tsz, 0:384],
                        )

                # ---- GCN: support (shifted token chunks) ------------
                x2T = actp.tile([P, KC, N], BF16, tag="actT")
                transpose_tm_to_fm(x2_sb, F32, id32, x2T)
                supp = actp.tile([P, 3, C], BF16, tag="vsm")
                for scidx, (ss, ssz) in enumerate(SH):
                    for half in range(2):
                        psu = ps1p.tile([P, 512], F32, tag="one")
                        for kc in range(KC):
                            nc.tensor.matmul(
                                psu[0:ssz, 0:384],
                                lhsT=x2T[:, kc, ss:ss + ssz],
                                rhs=wg[:, kc, half * 384:(half + 1) * 384],
                                start=(kc == 0),
                                stop=(kc == KC - 1),
                            )
                        if has_bias:
                            nc.vector.tensor_add(
                                out=psu[0:ssz, 0:384], in0=psu[0:ssz, 0:384],
                                in1=bgb[0:ssz, half * 384:(half + 1) * 384],
                            )
                        nc.any.tensor_copy(
                            out=supp[0:ssz, scidx, half * 384:(half + 1) * 384],
                            in_=psu[0:ssz, 0:384],
                        )

                # ---- GCN matmuls + LN3 + final out ------------------
                for scidx, (ss, ssz) in enumerate(SH):
                    y_sb = stp.tile([P, C], F32, tag="ych", bufs=1)
                    for half in range(2):
                        py = ps1p.tile([P, 512], F32, tag="one")
                        if scidx == 0:
                            # rows 0:64 = w_ts @ support[64:320]
                            for mm in range(2):
                                nc.tensor.matmul(
                                    py[0:64, 0:384],
                                    lhsT=wtsT[:, mm, :],
                                    rhs=supp[:, mm + 1, half * 384:(half + 1) * 384],
                                    start=(mm == 0),
                                    stop=(mm == 1),
                                )
                        else:
                            # rows 64+j = w_ts^T @ support[0:64]
                            nc.tensor.matmul(
                                py[0:ssz, 0:384],
                                lhsT=wts[:, (scidx - 1) * P:scidx * P],
                                rhs=supp[0:64, 0, half * 384:(half + 1) * 384],
                                start=True,
                                stop=True,
                            )
                        nc.any.tensor_copy(
                            out=y_sb[0:ssz, half * 384:(half + 1) * 384],
                            in_=py[0:ssz, 0:384],
                        )
                    # LN3 on this chunk
                    st6y = stp.tile([P, 3, 6], F32, tag="st6y")
                    for sub in range(3):
                        nc.vector.bn_stats(
                            out=st6y[0:ssz, sub, :],
                            in_=y_sb[0:ssz, sub * 256:(sub + 1) * 256],
                        )
                    mvy = stp.tile([P, 2], F32, tag="mvy")
                    nc.vector.bn_aggr(out=mvy[0:ssz, :], in_=st6y[0:ssz, :, :])
                    lnvy = stp.tile([P, 1], F32, tag="lnvy")
                    nc.scalar.activation(
                        out=lnvy[0:ssz, :], in_=mvy[0:ssz, 1:2], func=Ln,
                        bias=eps_t[0:ssz], scale=1.0,
                    )
                    rstdy = stp.tile([P, 1], F32, tag="rstdy")
                    nc.scalar.activation(
                        out=rstdy[0:ssz, :], in_=lnvy[0:ssz, :], func=Exp,
                        bias=0.0, scale=-0.5,
                    )
                    yfin = stp.tile([P, C], F32, tag="yfin", bufs=1)
                    nc.vector.tensor_scalar(
                        out=yfin[0:ssz, :], in0=y_sb[0:ssz, :],
                        scalar1=mvy[0:ssz, 0:1], scalar2=rstdy[0:ssz, 0:1],
                        op0=SUB, op1=MULT,
                    )
                    if has_aff3:
                        nc.vector.tensor_mul(
                            out=yfin[0:ssz, :], in0=yfin[0:ssz, :], in1=g3b[0:ssz, :]
                        )
                        nc.vector.tensor_add(
                            out=yfin[0:ssz, :], in0=yfin[0:ssz, :], in1=b3b[0:ssz, :]
                        )
                    nc.vector.tensor_add(
                        out=yfin[0:ssz, :], in0=yfin[0:ssz, :], in1=y_sb[0:ssz, :]
                    )
                    nc.sync.dma_start(out_e[s, ss:ss + ssz, :], yfin[0:ssz, :])

    nc.finalize()
    return nc


def _preprocess(inputs):
    """Host-side weight folding and dtype prep. Returns (weights_map, flags)."""
    f32 = np.float32
    g1 = np.asarray(inputs["g1"], f32)
    b1 = np.asarray(inputs["b1"], f32)
    g2 = np.asarray(inputs["g2"], f32)
    b2 = np.asarray(inputs["b2"], f32)
    wqkv = np.asarray(inputs["Wqkv"], f32)
    w1 = np.asarray(inputs["W1"], f32)

    wqkv_eff = g1[:, None] * wqkv
    bqkv_eff = b1 @ wqkv
    scale = HD ** (-0.5)
    wqkv_eff[:, 0:C] *= scale
    bqkv_eff[0:C] *= scale

    w1_eff = g2[:, None] * w1
    bm1_eff = np.asarray(inputs["bm1"], f32) + b2 @ w1

    bproj = np.asarray(inputs["bproj"], f32)
    bm2 = np.asarray(inputs["bm2"], f32)
    bg = np.asarray(inputs["bg"], f32)
    g3 = np.asarray(inputs["g3"], f32)
    b3 = np.asarray(inputs["b3"], f32)

    bexp = np.zeros((H, C), f32)
    for h in range(H):
        bexp[h, h * HD:(h + 1) * HD] = 1.0
    bsel = np.kron(np.eye(H, dtype=f32), np.ones((1, P), f32))

    bf = ml_dtypes.bfloat16
    wm = {
        "wqkv": wqkv_eff.astype(bf),
        "wproj": np.asarray(inputs["Wproj"], f32).astype(bf),
        "w1": w1_eff.astype(bf),
        "w2": np.asarray(inputs["W2"], f32).astype(bf),
        "wg": np.asarray(inputs["Wg"], f32).astype(bf),
        "bexp": bexp.astype(bf),
        "bsel": bsel.astype(bf),
        "id32": np.eye(P, dtype=f32),
        "id16": np.eye(P, dtype=f32).astype(bf),
    }
    flags = {}
    has_bias = any(
        np.abs(v).max() > 0 for v in (bqkv_eff, bm1_eff, bproj, bm2, bg)
    )
    flags["has_bias"] = bool(has_bias)
    if has_bias:
        wm["bqkv2"] = np.ascontiguousarray(
            bqkv_eff[: 2 * C].reshape(MC_QK, P).T).astype(f32)
        wm["bvv"] = np.ascontiguousarray(bqkv_eff[2 * C:]).astype(f32)
        wm["bm12"] = np.ascontiguousarray(bm1_eff.reshape(FC, P).T).astype(f32)
        wm["bprojv"] = bproj
        wm["bm2v"] = bm2
        wm["bgv"] = bg
    has_aff3 = bool(np.abs(g3 - 1.0).max() > 0 or np.abs(b3).max() > 0)
    flags["has_aff3"] = has_aff3
    if has_aff3:
        wm["g3v"] = g3
        wm["b3v"] = b3
    return wm, flags


_CACHE = {}


def _get_nc(flags, b_loc=B_LOC):
    key = (tuple(sorted(flags.items())), b_loc)
    if key not in _CACHE:
        _CACHE[key] = build_nc(b_loc=b_loc, flags=flags)
    return _CACHE[key]


def kernel(**inputs) -> np.ndarray:
    from concourse.bass_utils import run_bass_kernel_spmd

    x = np.ascontiguousarray(np.asarray(inputs["x"], np.float32))
    wm, flags = _preprocess(inputs)
    nc = _get_nc(flags)

    in_maps = []
    for i in range(N_CORES):
        m = dict(wm)
        m["x"] = np.ascontiguousarray(x[i * B_LOC:(i + 1) * B_LOC])
        in_maps.append(m)

    res = run_bass_kernel_spmd(nc, in_maps, core_ids=list(range(N_CORES)))
    out = np.concatenate([res.results[i]["out"] for i in range(N_CORES)], axis=0)
    return out.astype(np.float32)


# revision 37
# speedup vs baseline: 32522.2297x; 32522.2297x over previous
"""Trainium2 Bass kernel for nn_CEBlock (transformer block + attention-derived
GCN layer), data-parallel over 8 NeuronCores.

Self-contained: hardcodes all shapes; accepts the full unsharded inputs and
returns the full output.

Per-core program (B_LOC=4 samples), software-pipelined in emission order:
  A1(s): load x, LN1, transpose, QKV, logits+exp (key-major), ctx matmuls,
         denominator staging + reciprocal (runs on DVE/DMA under A1(s+1) PE)
  A2(s): ctx normalize (broadcast grid), w_ts adjacency softmax
  B(s):  proj+residual, LN2, MLP(+gelu), residual, GCN support + adjacency
         matmuls, LN3, final residual, store
Emission: A1(0), A1(1), then for s: A2(s), B(s), A1(s+2).
"""
import sys

if "/opt/trn_rl_repo" not in sys.path:
    sys.path.insert(0, "/opt/trn_rl_repo")

import numpy as np
import ml_dtypes

import concourse.bacc as bacc
import concourse.mybir as mybir
import concourse.tile as tile

F32 = mybir.dt.float32
BF16 = mybir.dt.bfloat16
FP8 = mybir.dt.float8e4

# Problem constants
B, N, C = 32, 320, 768
H, HD = 12, 64
LT, LS = 64, 256
MLP_H = 3072
EPS = 1e-5
N_CORES = 8
B_LOC = B // N_CORES

P = 128
KC = C // P
FC = MLP_H // P
MC_QK = (2 * C) // P
TCH = [(0, 128), (128, 128), (256, 64)]
SH = [(0, 64), (64, 128), (192, 128)]
KSZ = [128, 128, 64]

Gelu = mybir.ActivationFunctionType.Gelu
Exp = mybir.ActivationFunctionType.Exp
SUB = mybir.AluOpType.subtract
MULT = mybir.AluOpType.mult
ADD = mybir.AluOpType.add


def build_nc(b_loc=B_LOC, flags=None, repeat=1):
    fl = flags or {}
    nc = bacc.Bacc("TRN2", target_bir_lowering=False, debug=True)

    x_e = nc.dram_tensor("x", (b_loc, N, C), BF16, kind="ExternalInput")
    wqkv_e = nc.dram_tensor("wqkv", (C, 3 * C), BF16, kind="ExternalInput")
    wproj_e = nc.dram_tensor("wproj", (C, C), BF16, kind="ExternalInput")
    w1_e = nc.dram_tensor("w1", (C, MLP_H), BF16, kind="ExternalInput")
    w2_e = nc.dram_tensor("w2", (MLP_H, C), BF16, kind="ExternalInput")
    wg_e = nc.dram_tensor("wg", (C, C), BF16, kind="ExternalInput")
    bexp_e = nc.dram_tensor("bexp", (H, C), BF16, kind="ExternalInput")
    bsel_e = nc.dram_tensor("bsel", (H, H * P), BF16, kind="ExternalInput")
    id32_e = nc.dram_tensor("id32", (P, P), F32, kind="ExternalInput")
    id16_e = nc.dram_tensor("id16", (P, P), BF16, kind="ExternalInput")
    out_e = nc.dram_tensor("out", (b_loc, N, C), F32, kind="ExternalOutput")

    has_aff3 = fl.get("has_aff3", False)
    if has_aff3:
        g3_e = nc.dram_tensor("g3v", (C,), F32, kind="ExternalInput")
        b3_e = nc.dram_tensor("b3v", (C,), F32, kind="ExternalInput")
    has_bias = fl.get("has_bias", False)
    if has_bias:
        bqkv_e = nc.dram_tensor("bqkv2", (P, MC_QK), F32, kind="ExternalInput")
        bm1_e = nc.dram_tensor("bm12", (P, FC), F32, kind="ExternalInput")
        bv_e = nc.dram_tensor("bvv", (C,), F32, kind="ExternalInput")
        bproj_e = nc.dram_tensor("bprojv", (C,), F32, kind="ExternalInput")
        bm2_e = nc.dram_tensor("bm2v", (C,), F32, kind="ExternalInput")
        bg_e = nc.dram_tensor("bgv", (C,), F32, kind="ExternalInput")

    with tile.TileContext(nc) as tc:
        with (
            tc.tile_pool(name="wp", bufs=1) as wp,
            tc.tile_pool(name="act", bufs=1) as actp,
            tc.tile_pool(name="st", bufs=1) as stp,
            tc.tile_pool(name="ps", bufs=2, space="PSUM") as psp,
            tc.tile_pool(name="ps1", bufs=4, space="PSUM") as ps1p,
        ):
            # ---- small consts first (unblock sample-0 transposes) ----
            id32 = wp.tile([P, P], F32)
            nc.sync.dma_start(id32[:], id32_e[:])
            id16 = wp.tile([P, P], BF16)
            nc.sync.dma_start(id16[:], id16_e[:])
            bexp = wp.tile([H, KC, P], BF16)
            nc.sync.dma_start(bexp[:], bexp_e[:].rearrange("h (kc p) -> h kc p", p=P))
            bsel = wp.tile([H, H, P], BF16)
            nc.sync.dma_start(bsel[:], bsel_e[:].rearrange("a (h p) -> a h p", p=P))

            # ---- x prefetch helper ----------------------------------
            def load_x(s):
                x_sb = actp.tile([P, 3, C], BF16, tag="resid", bufs=4,
                                 name=f"x_{s}")
                nc.sync.dma_start(
                    x_sb[:, 0:2, :],
                    x_e[s, 0:256, :].rearrange("(t p) c -> p t c", p=P))
                nc.sync.dma_start(x_sb[0:64, 2, :], x_e[s, 256:320, :])
                return x_sb

            # ---- helpers --------------------------------------------
            def newton_rsqrt(dst, var_ap, nf):
                pdim = dst.shape[0]
                v = stp.tile([P, 4], F32, tag="nw_v", bufs=2)
                t = stp.tile([P, 4], F32, tag="nw_t", bufs=2)
                nc.vector.tensor_scalar(
                    out=v[0:pdim, 0:nf], in0=var_ap, scalar1=EPS,
                    scalar2=None, op0=ADD)
                nc.vector.tensor_scalar(
                    out=dst[0:pdim, 0:nf].bitcast(mybir.dt.int32),
                    in0=v[0:pdim, 0:nf].bitcast(mybir.dt.int32),
                    scalar1=1, scalar2=None,
                    op0=mybir.AluOpType.logical_shift_right)
                nc.vector.tensor_scalar(
                    out=dst[0:pdim, 0:nf].bitcast(mybir.dt.int32),
                    in0=dst[0:pdim, 0:nf].bitcast(mybir.dt.int32),
                    scalar1=0x5F3759DF, scalar2=-1, op0=SUB, op1=MULT)
                for _ in range(2):
                    nc.vector.tensor_mul(out=t[0:pdim, 0:nf],
                                         in0=dst[0:pdim, 0:nf],
                                         in1=dst[0:pdim, 0:nf])
                    nc.vector.tensor_mul(out=t[0:pdim, 0:nf],
                                         in0=t[0:pdim, 0:nf],
                                         in1=v[0:pdim, 0:nf])
                    nc.vector.tensor_scalar(
                        out=t[0:pdim, 0:nf], in0=t[0:pdim, 0:nf],
                        scalar1=-0.5, scalar2=1.5, op0=MULT, op1=ADD)
                    nc.vector.tensor_mul(out=dst[0:pdim, 0:nf],
                                         in0=dst[0:pdim, 0:nf],
                                         in1=t[0:pdim, 0:nf])

            def layernorm_to(src, out_tile):
                st6 = stp.tile([P, 3, 3, 6], F32, tag="st6", bufs=2)
                mv3 = stp.tile([P, 3, 2], F32, tag="mv3", bufs=2)
                nc.vector.memset(mv3[:], 1.0)
                for tcidx, (ts, tsz) in enumerate(TCH):
                    for sub in range(3):
                        nc.vector.bn_stats(
                            out=st6[0:tsz, tcidx, sub, :],
                            in_=src[0:tsz, tcidx, sub * 256:(sub + 1) * 256])
                    nc.vector.bn_aggr(
                        out=mv3[0:tsz, tcidx, :], in_=st6[0:tsz, tcidx, :, :])
                rstd3 = stp.tile([P, 3], F32, tag="rstd3", bufs=2)
                newton_rsqrt(rstd3, mv3[:, :, 1], 3)
                for tcidx, (ts, tsz) in enumerate(TCH):
                    nc.vector.tensor_scalar(
                        out=out_tile[0:tsz, tcidx, :],
                        in0=src[0:tsz, tcidx, :],
                        scalar1=mv3[0:tsz, tcidx, 0:1],
                        scalar2=rstd3[0:tsz, tcidx:tcidx + 1],
                        op0=SUB, op1=MULT)

            def transpose_tm_to_fm(src_tm, ident, out_fm):
                nch = src_tm.shape[2] // P
                for kc in range(nch):
                    pt = ps1p.tile([P, 512], BF16, tag="one")
                    for tcidx, (ts, tsz) in enumerate(TCH):
                        nc.tensor.transpose(
                            pt[:, ts:ts + tsz],
                            src_tm[0:tsz, tcidx, kc * P:(kc + 1) * P],
                            ident[0:tsz, 0:tsz])
                    nc.any.tensor_copy(out=out_fm[:, kc, :], in_=pt[:, 0:N])

            # =========================================================
            def phase_pre(s):
                x_sb = load_x(s % b_loc)
                st = {"x": x_sb}
                xn_tm = actp.tile([P, 3, C], BF16, tag="xn", bufs=2)
                layernorm_to(x_sb, xn_tm)
                st["xn"] = xn_tm
                return st

            def phase_a1(s, st):
                x_sb, xn_tm = st["x"], st["xn"]
                xnT = actp.tile([P, KC, N], BF16, tag="actT", bufs=2)
                transpose_tm_to_fm(xn_tm, id16, xnT)

                qk = actp.tile([P, MC_QK, N], BF16, tag="qk")
                for mc in range(MC_QK):
                    pq = ps1p.tile([P, 512], F32, tag="one")
                    for kc in range(KC):
                        nc.tensor.matmul(
                            pq[:, 0:N],
                            lhsT=wqkv[:, kc, mc * P:(mc + 1) * P],
                            rhs=xnT[:, kc, :],
                            start=(kc == 0), stop=(kc == KC - 1))
                    if has_bias:
                        nc.vector.tensor_scalar(
                            out=qk[:, mc, :], in0=pq[:, 0:N],
                            scalar1=bqkv[:, mc:mc + 1], scalar2=None, op0=ADD)
                    else:
                        nc.any.tensor_copy(out=qk[:, mc, :], in_=pq[:, 0:N])

                v_sb = actp.tile([P, 3, H, HD + 1], BF16, tag="vsm")
                for tcidx, (ts, tsz) in enumerate(TCH):
                    for half in range(2):
                        pv = ps1p.tile([P, 512], F32, tag="one")
                        for kc in range(KC):
                            nc.tensor.matmul(
                                pv[0:tsz, 0:384],
                                lhsT=xnT[:, kc, ts:ts + tsz],
                                rhs=wqkv[:, kc,
                                         2 * C + half * 384:2 * C + (half + 1) * 384],
                                start=(kc == 0), stop=(kc == KC - 1))
                        if has_bias:
                            nc.vector.tensor_add(
                                out=pv[0:tsz, 0:384], in0=pv[0:tsz, 0:384],
                                in1=bvb[0:tsz, half * 384:(half + 1) * 384])
                        nc.any.tensor_copy(
                            out=v_sb[0:tsz, tcidx, half * 6:(half + 1) * 6, 0:HD],
                            in_=pv[0:tsz, 0:384].rearrange(
                                "p (h d) -> p h d", d=HD))
                    nc.vector.memset(v_sb[0:tsz, tcidx, :, HD:HD + 1], 1.0)

                probs = actp.tile([P, 3, H, N], BF16, tag="big")
                for j, kz in enumerate(KSZ):
                    for g in range(H // 2):
                        pl = psp.tile([P, 2, 512], F32, tag="pair")
                        for hh in range(2):
                            h = 2 * g + hh
                            po = (h % 2) * HD
                            nc.tensor.matmul(
                                pl[0:kz, hh, 0:N],
                                lhsT=qk[po:po + HD, 6 + h // 2,
                                        TCH[j][0]:TCH[j][0] + kz],
                                rhs=qk[po:po + HD, h // 2, :],
                                start=True, stop=True)
                        nc.scalar.activation(
                            out=probs[0:kz, j, 2 * g:2 * g + 2, :],
                            in_=pl[0:kz, :, 0:N], func=Exp)

                # small template-column copy so probs can be released at
                # the end of A1 (w_ts consumes this in A2 instead).
                # packed: slot0 = j1 (128 rows); slot1 = j0@rows0:64 +
                # j2@rows64:128 (DVE handles cross-quadrant bases)
                pw_sb = actp.tile([P, 2, H, LT], BF16, tag="pwts", bufs=2)
                nc.vector.tensor_copy(
                    out=pw_sb[:, 0, :, :], in_=probs[:, 1, :, 0:LT])
                nc.vector.tensor_copy(
                    out=pw_sb[0:64, 1, :, :], in_=probs[64:128, 0, :, 0:LT])
                nc.vector.tensor_copy(
                    out=pw_sb[64:128, 1, :, :], in_=probs[0:64, 2, :, 0:LT])

                ctx = actp.tile([P, KC, N], BF16, tag="ctx", bufs=2)
                dstage = stp.tile([H, N], F32, tag="dstage")
                for h in range(H):
                    pc = ps1p.tile([HD + 1, 512], F32, tag="one")
                    for j, kz in enumerate(KSZ):
                        nc.tensor.matmul(
                            pc[:, 0:N],
                            lhsT=v_sb[0:kz, j, h, :],
                            rhs=probs[0:kz, j, h, :],
                            start=(j == 0), stop=(j == 2))
                    nc.vector.tensor_copy(
                        out=ctx[(h % 2) * HD:(h % 2) * HD + HD, h // 2, :],
                        in_=pc[0:HD, 0:N])
                    d64 = stp.tile([HD + 1, N], F32, tag="d64", bufs=2)
                    nc.any.tensor_copy(
                        out=d64[HD:HD + 1, :], in_=pc[HD:HD + 1, 0:N])
                    nc.sync.dma_start(dstage[h:h + 1, :], d64[HD:HD + 1, :])
                recf = stp.tile([H, N], F32, tag="recf")
                nc.vector.reciprocal(out=recf[:], in_=dstage[:])
                recip = stp.tile([H, N], BF16, tag="recip", bufs=2)
                nc.vector.tensor_copy(out=recip[:], in_=recf[:])
                st.update(ctx=ctx, pw_sb=pw_sb, recip=recip)
                return st

            def phase_a2_grid(s, st):
                ctx, recip, pw_sb = st["ctx"], st["recip"], st["pw_sb"]
                # normalize ctx in-place via broadcast grid
                for kc in range(KC):
                    pg = ps1p.tile([P, 512], F32, tag="one")
                    nc.tensor.matmul(pg[:, 0:N], lhsT=bexp[:, kc, :],
                                     rhs=recip[:], start=True, stop=True)
                    nc.vector.tensor_mul(
                        out=ctx[:, kc, :], in0=ctx[:, kc, :], in1=pg[:, 0:N])
                # w_ts pieces (DVE work runs under the proj matmuls)
                prk = psp.tile([P, 2, 512], F32, tag="pair")
                for h in range(H):
                    fo = h * HD
                    nc.tensor.matmul(
                        prk[:, fo // 512, fo % 512:fo % 512 + HD],
                        lhsT=bsel[:, h, :], rhs=recip[:, 0:LT],
                        start=True, stop=True)
                prk_v = prk[:].rearrange("p a b -> p (a b)")[:, 0:H * HD] \
                    .rearrange("p (h i) -> p h i", i=HD)
                wtmp = stp.tile([P, H, LT], BF16, tag="wtmp")
                wpiece = stp.tile([P, 3, LT], F32, tag="wpiece")
                for j, (slot, po, kz) in enumerate(
                        [(1, 0, 64), (0, 0, 128), (1, 64, 64)]):
                    nc.vector.tensor_tensor(
                        out=wtmp[po:po + kz, :, :],
                        in0=pw_sb[po:po + kz, slot, :, :],
                        in1=prk_v[po:po + kz, :, :], op=MULT)
                    nc.vector.tensor_reduce(
                        out=wpiece[0:kz, j, :],
                        in_=wtmp[po:po + kz, :, :].rearrange("p h i -> p i h"),
                        axis=mybir.AxisListType.X, op=ADD)
                st["wpiece"] = wpiece

            def phase_a2_wts(s, st):
                wpiece = st["wpiece"]
                pw = ps1p.tile([64, 512], F32, tag="one")
                nc.tensor.transpose(pw[:, 0:64], wpiece[0:64, 0, :],
                                    id32[0:64, 0:64])
                nc.tensor.transpose(pw[:, 64:192], wpiece[0:128, 1, :],
                                    id32[:, :])
                nc.tensor.transpose(pw[:, 192:256], wpiece[0:64, 2, :],
                                    id32[0:64, 0:64])
                wts_e = stp.tile([LT, LS], BF16, tag="wts_e")
                wden = stp.tile([LT, 1], F32, tag="wden", bufs=2)
                nc.scalar.activation(out=wts_e[:], in_=pw[:, 0:LS], func=Exp,
                                     scale=1.0 / H, accum_out=wden[:])
                wrec = stp.tile([LT, 1], F32, tag="wrec", bufs=2)
                nc.vector.reciprocal(out=wrec[:], in_=wden[:])
                wts = stp.tile([LT, LS], BF16, tag="wts", bufs=2)
                nc.vector.tensor_scalar_mul(wts[:], wts_e[:], wrec[:])
                wtsT = stp.tile([P, 2, LT], BF16, tag="wtsT", bufs=2)
                for mm in range(2):
                    pwt = ps1p.tile([P, 512], BF16, tag="one")
                    nc.tensor.transpose(pwt[:, 0:LT],
                                        wts[:, mm * P:(mm + 1) * P],
                                        id16[0:LT, 0:LT])
                    nc.any.tensor_copy(out=wtsT[:, mm, :], in_=pwt[:, 0:LT])
                st.update(wts=wts, wtsT=wtsT)

            def phase_b_proj(s, st):
                x_sb, ctxn = st["x"], st["ctx"]
                x1_sb = actp.tile([P, 3, C], BF16, tag="resid", bufs=4)
                st6 = stp.tile([P, 3, 3, 6], F32, tag="st6", bufs=2)
                mv3 = stp.tile([P, 3, 2], F32, tag="mv3", bufs=2)
                nc.vector.memset(mv3[:], 1.0)
                for tcidx, (ts, tsz) in enumerate(TCH):
                    for half in range(2):
                        pp = ps1p.tile([P, 512], F32, tag="one")
                        for kc in range(KC):
                            nc.tensor.matmul(
                                pp[0:tsz, 0:384],
                                lhsT=ctxn[:, kc, ts:ts + tsz],
                                rhs=wproj[:, kc, half * 384:(half + 1) * 384],
                                start=(kc == 0), stop=(kc == KC - 1))
                        if has_bias:
                            nc.vector.tensor_add(
                                out=pp[0:tsz, 0:384], in0=pp[0:tsz, 0:384],
                                in1=bprojb[0:tsz, half * 384:(half + 1) * 384])
                        nc.vector.tensor_add(
                            out=x1_sb[0:tsz, tcidx, half * 384:(half + 1) * 384],
                            in0=x_sb[0:tsz, tcidx, half * 384:(half + 1) * 384],
                            in1=pp[0:tsz, 0:384])
                    # LN2 stats interleaved per chunk
                    for sub in range(3):
                        nc.vector.bn_stats(
                            out=st6[0:tsz, tcidx, sub, :],
                            in_=x1_sb[0:tsz, tcidx, sub * 256:(sub + 1) * 256])
                    nc.vector.bn_aggr(
                        out=mv3[0:tsz, tcidx, :], in_=st6[0:tsz, tcidx, :, :])
                rstd3 = stp.tile([P, 3], F32, tag="rstd3", bufs=2)
                newton_rsqrt(rstd3, mv3[:, :, 1], 3)
                xn2_tm = actp.tile([P, 3, C], BF16, tag="xn", bufs=2)
                for tcidx, (ts, tsz) in enumerate(TCH):
                    nc.vector.tensor_scalar(
                        out=xn2_tm[0:tsz, tcidx, :],
                        in0=x1_sb[0:tsz, tcidx, :],
                        scalar1=mv3[0:tsz, tcidx, 0:1],
                        scalar2=rstd3[0:tsz, tcidx:tcidx + 1],
                        op0=SUB, op1=MULT)
                st.update(x1=x1_sb, xn2=xn2_tm)

            def phase_b_rest(s, st):
                x1_sb, xn2_tm = st["x1"], st["xn2"]
                wts, wtsT = st["wts"], st["wtsT"]
                xn2T = actp.tile([P, KC, N], BF16, tag="actT", bufs=2)
                transpose_tm_to_fm(xn2_tm, id16, xn2T)

                h_sb = actp.tile([P, FC, N], BF16, tag="big")
                for q in range(4):  # stream W1 in quarters (saves SBUF)
                    w1q = actp.tile([P, KC, 6 * P], BF16, tag="w1q", bufs=2)
                    nc.sync.dma_start(
                        w1q[:],
                        w1_e[:, q * 6 * P:(q + 1) * 6 * P].rearrange(
                            "(kc p) m -> p kc m", p=P))
                    for g3 in range(3):
                        g = q * 3 + g3
                        ph = psp.tile([P, 2, 512], F32, tag="pair")
                        for ff in range(2):
                            fcl = g3 * 2 + ff
                            for kc in range(KC):
                                nc.tensor.matmul(
                                    ph[:, ff, 0:N],
                                    lhsT=w1q[:, kc, fcl * P:(fcl + 1) * P],
                                    rhs=xn2T[:, kc, :],
                                    start=(kc == 0), stop=(kc == KC - 1))
                        if has_bias:
                            for ff in range(2):
                                nc.scalar.activation(
                                    out=h_sb[:, 2 * g + ff, :],
                                    in_=ph[:, ff, 0:N], func=Gelu,
                                    bias=bm1[:, 2 * g + ff:2 * g + ff + 1],
                                    scale=1.0)
                        else:
                            nc.scalar.activation(
                                out=h_sb[:, 2 * g:2 * g + 2, :],
                                in_=ph[:, :, 0:N], func=Gelu)

                # MLP2 feature-major (46k vs 55k cycles), then transpose back
                m2T = actp.tile([P, KC, N], BF16, tag="actT", bufs=2)
                for cc in range(KC):
                    pm = ps1p.tile([P, 512], F32, tag="one")
                    for fc in range(FC):
                        nc.tensor.matmul(
                            pm[:, 0:N],
                            lhsT=w2[:, fc, cc * P:(cc + 1) * P],
                            rhs=h_sb[:, fc, :],
                            start=(fc == 0), stop=(fc == FC - 1))
                    if has_bias:
                        nc.vector.tensor_scalar(
                            out=m2T[:, cc, :], in0=pm[:, 0:N],
                            scalar1=bm2p[:, cc:cc + 1], scalar2=None, op0=ADD)
                    else:
                        nc.any.tensor_copy(out=m2T[:, cc, :], in_=pm[:, 0:N])
                x2_sb = actp.tile([P, 3, C], BF16, tag="resid", bufs=4)
                for tcidx, (ts, tsz) in enumerate(TCH):
                    pa = ps1p.tile([P, 512], BF16, tag="one")
                    pb = ps1p.tile([P, 512], BF16, tag="one")
                    for cc in range(KC):
                        dst = pa[0:tsz, cc * P:(cc + 1) * P] if cc < 4 else                             pb[0:tsz, (cc - 4) * P:(cc - 3) * P]
                        nc.tensor.transpose(
                            dst, m2T[:, cc, ts:ts + tsz], id16[:, :])
                    nc.vector.tensor_add(
                        out=x2_sb[0:tsz, tcidx, 0:512],
                        in0=x1_sb[0:tsz, tcidx, 0:512], in1=pa[0:tsz, 0:512])
                    nc.vector.tensor_add(
                        out=x2_sb[0:tsz, tcidx, 512:768],
                        in0=x1_sb[0:tsz, tcidx, 512:768], in1=pb[0:tsz, 0:256])

                x2T = actp.tile([P, KC, N], BF16, tag="actT", bufs=2)
                transpose_tm_to_fm(x2_sb, id16, x2T)
                supp = actp.tile([P, 3, C], BF16, tag="vsm")
                for scidx, (ss, ssz) in enumerate(SH):
                    for half in range(2):
                        psu = ps1p.tile([P, 512], F32, tag="one")
                        for kc in range(KC):
                            nc.tensor.matmul(
                                psu[0:ssz, 0:384],
                                lhsT=x2T[:, kc, ss:ss + ssz],
                                rhs=wg[:, kc, half * 384:(half + 1) * 384],
                                start=(kc == 0), stop=(kc == KC - 1))
                        if has_bias:
                            nc.vector.tensor_add(
                                out=psu[0:ssz, 0:384], in0=psu[0:ssz, 0:384],
                                in1=bgb[0:ssz, half * 384:(half + 1) * 384])
                        nc.any.tensor_copy(
                            out=supp[0:ssz, scidx, half * 384:(half + 1) * 384],
                            in_=psu[0:ssz, 0:384])

                for scidx, (ss, ssz) in enumerate(SH):
                    y_sb = stp.tile([P, C], F32, tag="ych", bufs=2)
                    for half in range(2):
                        py = ps1p.tile([P, 512], F32, tag="one")
                        if scidx == 0:
                            for mm in range(2):
                                nc.tensor.matmul(
                                    py[0:64, 0:384],
                                    lhsT=wtsT[:, mm, :],
                                    rhs=supp[:, mm + 1,
                                             half * 384:(half + 1) * 384],
                                    start=(mm == 0), stop=(mm == 1))
                        else:
                            nc.tensor.matmul(
                                py[0:ssz, 0:384],
                                lhsT=wts[:, (scidx - 1) * P:scidx * P],
                                rhs=supp[0:64, 0, half * 384:(half + 1) * 384],
                                start=True, stop=True)
                        nc.any.tensor_copy(
                            out=y_sb[0:ssz, half * 384:(half + 1) * 384],
                            in_=py[0:ssz, 0:384])
                    st6y = stp.tile([P, 3, 6], F32, tag="st6y", bufs=2)
                    for sub in range(3):
                        nc.vector.bn_stats(
                            out=st6y[0:ssz, sub, :],
                            in_=y_sb[0:ssz, sub * 256:(sub + 1) * 256])
                    mvy = stp.tile([P, 2], F32, tag="mvy", bufs=2)
                    nc.vector.bn_aggr(out=mvy[0:ssz, :], in_=st6y[0:ssz, :, :])
                    rstdy = stp.tile([P, 1], F32, tag="rstdy", bufs=2)
                    newton_rsqrt(rstdy[0:ssz], mvy[0:ssz, 1:2], 1)
                    yfin = stp.tile([P, C], F32, tag="yfin")
                    nc.vector.tensor_scalar(
                        out=yfin[0:ssz, :], in0=y_sb[0:ssz, :],
                        scalar1=mvy[0:ssz, 0:1], scalar2=rstdy[0:ssz, 0:1],
                        op0=SUB, op1=MULT)
                    if has_aff3:
                        nc.vector.tensor_mul(out=yfin[0:ssz, :],
                                             in0=yfin[0:ssz, :],
                                             in1=g3b[0:ssz, :])
                        nc.vector.tensor_add(out=yfin[0:ssz, :],
                                             in0=yfin[0:ssz, :],
                                             in1=b3b[0:ssz, :])
                    nc.vector.tensor_add(out=yfin[0:ssz, :],
                                         in0=yfin[0:ssz, :],
                                         in1=y_sb[0:ssz, :])
                    nc.sync.dma_start(out_e[s, ss:ss + ssz, :], yfin[0:ssz, :])

            # ---- software-pipelined emission ------------------------
            total = b_loc * repeat
            states = {}
            states[0] = phase_pre(0)
            if total > 1:
                states[1] = phase_pre(1)
            # ---- weights (in order of first use) --------------------
            wqkv = wp.tile([P, KC, 3 * C], BF16)
            for kc in range(KC):
                nc.sync.dma_start(
                    wqkv[:, kc, :],
                    wqkv_e[kc * P:(kc + 1) * P, :])
            wproj = wp.tile([P, KC, C], BF16)
            for kc in range(KC):
                nc.sync.dma_start(
                    wproj[:, kc, :], wproj_e[kc * P:(kc + 1) * P, :])
            w2 = wp.tile([P, FC, C], BF16)
            for fc in range(0, FC, 4):
                nc.sync.dma_start(
                    w2[:, fc:fc + 4, :],
                    w2_e[fc * P:(fc + 4) * P, :].rearrange(
                        "(fc p) m -> p fc m", p=P))
            wg = wp.tile([P, KC, C], BF16)
            for kc in range(KC):
                nc.sync.dma_start(
                    wg[:, kc, :], wg_e[kc * P:(kc + 1) * P, :])
            if has_aff3:
                g3b = wp.tile([P, C], F32)
                nc.sync.dma_start(g3b[:], g3_e[None, :].to_broadcast((P, C)))
                b3b = wp.tile([P, C], F32)
                nc.sync.dma_start(b3b[:], b3_e[None, :].to_broadcast((P, C)))
            if has_bias:
                bqkv = wp.tile([P, MC_QK], F32)
                nc.sync.dma_start(bqkv[:], bqkv_e[:])
                bm1 = wp.tile([P, FC], F32)
                nc.sync.dma_start(bm1[:], bm1_e[:])
                bvb = wp.tile([P, C], F32)
                nc.sync.dma_start(bvb[:], bv_e[None, :].to_broadcast((P, C)))
                bprojb = wp.tile([P, C], F32)
                nc.sync.dma_start(bprojb[:], bproj_e[None, :].to_broadcast((P, C)))
                bm2p = wp.tile([P, KC], F32)
                nc.sync.dma_start(
                    bm2p[:], bm2_e[:].rearrange("(kc p) -> p kc", p=P))
                bgb = wp.tile([P, C], F32)
                nc.sync.dma_start(bgb[:], bg_e[None, :].to_broadcast((P, C)))


            phase_a1(0, states[0])
            if total > 1:
                phase_a1(1, states[1])
            for i in range(total):
                phase_a2_grid(i, states[i])
                phase_b_proj(i % b_loc, states[i])
                phase_a2_wts(i, states[i])
                if i + 2 < total:
                    states[i + 2] = phase_pre(i + 2)
                phase_b_rest(i % b_loc, states[i])
                del states[i]
                if i + 2 < total:
                    phase_a1(i + 2, states[i + 2])

    nc.finalize()
    return nc


def _preprocess(inputs):
    f32 = np.float32
    g1 = np.asarray(inputs["g1"], f32)
    b1 = np.asarray(inputs["b1"], f32)
    g2 = np.asarray(inputs["g2"], f32)
    b2 = np.asarray(inputs["b2"], f32)
    wqkv = np.asarray(inputs["Wqkv"], f32)
    w1 = np.asarray(inputs["W1"], f32)

    wqkv_eff = g1[:, None] * wqkv
    bqkv_eff = b1 @ wqkv
    scale = HD ** (-0.5)
    wqkv_eff[:, 0:C] *= scale
    bqkv_eff[0:C] *= scale

    w1_eff = g2[:, None] * w1
    bm1_eff = np.asarray(inputs["bm1"], f32) + b2 @ w1

    bproj = np.asarray(inputs["bproj"], f32)
    bm2 = np.asarray(inputs["bm2"], f32)
    bg = np.asarray(inputs["bg"], f32)
    g3 = np.asarray(inputs["g3"], f32)
    b3 = np.asarray(inputs["b3"], f32)

    bexp = np.zeros((H, C), f32)
    for h in range(H):
        bexp[h, h * HD:(h + 1) * HD] = 1.0
    bsel = np.kron(np.eye(H, dtype=f32), np.ones((1, P), f32))

    bf = ml_dtypes.bfloat16
    wm = {
        "wqkv": wqkv_eff.astype(bf),
        "wproj": np.asarray(inputs["Wproj"], f32).astype(bf),
        "w1": w1_eff.astype(bf),
        "w2": np.asarray(inputs["W2"], f32).astype(bf),
        "wg": np.asarray(inputs["Wg"], f32).astype(bf),
        "bexp": bexp.astype(bf),
        "bsel": bsel.astype(bf),
        "id32": np.eye(P, dtype=f32),
        "id16": np.eye(P, dtype=f32).astype(bf),
    }
    flags = {}
    has_bias = any(
        np.abs(v).max() > 0 for v in (bqkv_eff, bm1_eff, bproj, bm2, bg))
    flags["has_bias"] = bool(has_bias)
    if has_bias:
        wm["bqkv2"] = np.ascontiguousarray(
            bqkv_eff[: 2 * C].reshape(MC_QK, P).T).astype(f32)
        wm["bvv"] = np.ascontiguousarray(bqkv_eff[2 * C:]).astype(f32)
        wm["bm12"] = np.ascontiguousarray(bm1_eff.reshape(FC, P).T).astype(f32)
        wm["bprojv"] = bproj
        wm["bm2v"] = bm2
        wm["bgv"] = bg
    has_aff3 = bool(np.abs(g3 - 1.0).max() > 0 or np.abs(b3).max() > 0)
    flags["has_aff3"] = has_aff3
    if has_aff3:
        wm["g3v"] = g3
        wm["b3v"] = b3
    return wm, flags


_CACHE = {}


def _get_nc(flags, b_loc=B_LOC):
    key = (tuple(sorted(flags.items())), b_loc)
    if key not in _CACHE:
        _CACHE[key] = build_nc(b_loc=b_loc, flags=flags)
    return _CACHE[key]


def kernel(**inputs) -> np.ndarray:
    from concourse.bass_utils import run_bass_kernel_spmd

    x = np.ascontiguousarray(
        np.asarray(inputs["x"], np.float32)).astype(ml_dtypes.bfloat16)
    wm, flags = _preprocess(inputs)
    nc = _get_nc(flags)

    in_maps = []
    for i in range(N_CORES):
        m = dict(wm)
        m["x"] = np.ascontiguousarray(x[i * B_LOC:(i + 1) * B_LOC])
        in_maps.append(m)

    res = run_bass_kernel_spmd(nc, in_maps, core_ids=list(range(N_CORES)))
    out = np.concatenate([res.results[i]["out"] for i in range(N_CORES)], axis=0)
    return out.astype(np.float32)
